# revision 9
# baseline (speedup 1.0000x reference)
"""Bidirectional GRU duration predictor on 8 Trainium2 NeuronCores.

Sharding: core c in 0..7 handles direction d = c//4 (0=fwd, 1=bwd, bwd cores
get time-reversed features) and batch rows [8*(c%4), 8*(c%4)+8).  All cores
run the same Bass program on different data.

Per-core device layout (transposed everywhere; state h kept as hT [128, 2, 8]
= [H-chunk partitions, K-chunk, batch]):
  - gi = Wi.T @ feats precomputed on-chip in 128-step chunks (PE), stored bf16.
  - scan step: PSUM <- identity-preload of gi(r,z) / bhn, then 12 bf16 Wh-tile
    matmuls accumulate Wh.T @ h; sigmoid/tanh on ACT; fused DVE ops produce
    h_new (fp32) + bf16 copy for the next matmul.
  - output projection h . Wd_half accumulated per-step into PSUM via 2 tiny
    matmuls, DMA'd to DRAM every 64 steps.
Host reassembles out = fwd_part + bwd_part + bd.
"""

import sys

if "/opt/trn_rl_repo" not in sys.path:
    sys.path.insert(0, "/opt/trn_rl_repo")

import numpy as np
import ml_dtypes

import concourse.bacc as bacc
import concourse.tile as tile
import concourse.mybir as mybir
from concourse.bass_utils import run_bass_kernel_spmd
from concourse.masks import make_identity

BF16 = mybir.dt.bfloat16
F32 = mybir.dt.float32
NPBF16 = ml_dtypes.bfloat16
AF = mybir.ActivationFunctionType
OP = mybir.AluOpType

B, T_FULL, H, FEAT = 32, 2048, 256, 64
NCORES = 8
SHARDS = 4          # batch shards per direction
BC = B // SHARDS    # 8 batch rows per core
GI_PIECES = 12      # 6 M-chunks x 2 halves per gi chunk


def build_program(T=T_FULL, tc=128):
    assert T % tc == 0
    n_chunks = T // tc
    nc = bacc.Bacc()

    feats_d = nc.dram_tensor("featsT", [FEAT, T * BC], BF16, kind="ExternalInput")
    whb_d = nc.dram_tensor("whb", [128, 2 * 768], BF16, kind="ExternalInput")
    wib_d = nc.dram_tensor("wib", [FEAT, 768], BF16, kind="ExternalInput")
    bi_d = nc.dram_tensor("bi6", [128, 6], F32, kind="ExternalInput")
    bhnr_d = nc.dram_tensor("bhnr", [128, 2 * BC], BF16, kind="ExternalInput")
    wd_d = nc.dram_tensor("wd2", [128, 2], BF16, kind="ExternalInput")
    y_d = nc.dram_tensor("y", [1, T * BC], F32, kind="ExternalOutput")

    with tile.TileContext(nc) as tcx:
        with (
            tcx.tile_pool(name="persist", bufs=1) as persist,
            tcx.tile_pool(name="gates", bufs=8) as gates,
            tcx.tile_pool(name="ps_rz", bufs=2, space="PSUM") as ps_rz,
            tcx.tile_pool(name="ps_n", bufs=2, space="PSUM") as ps_n,
            tcx.tile_pool(name="ps_out", bufs=2, space="PSUM") as ps_out,
            tcx.tile_pool(name="ps_gi", bufs=2, space="PSUM") as ps_gi,
        ):
            feats_s = persist.tile([FEAT, T * BC], BF16, tag="feats")
            whb_s = persist.tile([128, 2 * 768], BF16, tag="whb")
            wib_s = persist.tile([FEAT, 768], BF16, tag="wib")
            bi_s = persist.tile([128, 6], F32, tag="bi")
            bhnr_s = persist.tile([128, 2 * BC], BF16, tag="bhnr")
            wd_s = persist.tile([128, 2], BF16, tag="wd")
            ident = persist.tile([128, 128], BF16, tag="ident")
            hT = persist.tile([128, 2 * BC], F32, tag="hT")
            h_bf = persist.tile([128, 2 * BC], BF16, tag="h_bf")
            gi_buf0 = persist.tile([128, tc, 6, BC], BF16, tag="gi0")
            gi_buf1 = persist.tile([128, tc, 6, BC], BF16, tag="gi1")
            gi_bufs = [gi_buf0, gi_buf1]

            # ---- prologue: parameter DMAs, identity, zero state ----
            nc.sync.dma_start(whb_s[:], whb_d[:])
            nc.sync.dma_start(wib_s[:], wib_d[:])
            nc.sync.dma_start(bi_s[:], bi_d[:])
            nc.sync.dma_start(bhnr_s[:], bhnr_d[:])
            nc.sync.dma_start(wd_s[:], wd_d[:])
            for c in range(n_chunks):
                nc.sync.dma_start(
                    feats_s[:, c * tc * BC : (c + 1) * tc * BC],
                    feats_d[:, c * tc * BC : (c + 1) * tc * BC],
                )
            make_identity(nc, ident[:])
            nc.gpsimd.memset(hT[:], 0.0)
            nc.gpsimd.memset(h_bf[:], 0.0)

            def emit_gi_piece(c, idx):
                """One (matmul, biased-copy) pair of gi chunk c: piece idx."""
                dst = gi_bufs[c % 2]
                mc, half = idx // 2, idx % 2
                hw = tc * BC // 2  # columns per half
                gps = ps_gi.tile([128, hw], F32, tag="gips")
                col0 = c * tc * BC + half * hw
                nc.tensor.matmul(
                    gps[:, :],
                    lhsT=wib_s[:, mc * 128 : (mc + 1) * 128],
                    rhs=feats_s[:, col0 : col0 + hw],
                    start=True,
                    stop=True,
                    skip_group_check=True,
                )
                dvec = dst[:, half * (tc // 2) : (half + 1) * (tc // 2), mc, :]
                nc.vector.tensor_scalar_add(
                    dvec, gps[:].rearrange("p (t b) -> p t b", b=BC), bi_s[:, mc : mc + 1]
                )

            state = {"out_ps": None}

            def proj_prev(t):
                """Project ys[t-1] (current h_bf) into the output PSUM strip."""
                j = (t - 1) % 64
                if j == 0:
                    state["out_ps"] = ps_out.tile(
                        [1, 512], F32, tag="outps", name="outps"
                    )
                op = state["out_ps"]
                nc.tensor.matmul(
                    op[:, j * 8 : j * 8 + 8],
                    lhsT=wd_s[:, 0:1],
                    rhs=h_bf[:, 0:BC],
                    start=True,
                    stop=False,
                    skip_group_check=True,
                )
                nc.tensor.matmul(
                    op[:, j * 8 : j * 8 + 8],
                    lhsT=wd_s[:, 1:2],
                    rhs=h_bf[:, BC : 2 * BC],
                    start=False,
                    stop=True,
                    skip_group_check=True,
                )
                if j == 63:
                    ysb = gates.tile([1, 512], F32, tag="ysb", name="ysb")
                    nc.vector.tensor_copy(ysb[:], op[:, :])
                    nc.sync.dma_start(y_d[0:1, (t - 64) * 8 : t * 8], ysb[:])

            def emit_step(t):
                c, tloc = t // tc, t % tc
                gi_cur = gi_bufs[c % 2]

                ghrz = ps_rz.tile([128, 4 * BC], F32, tag="ghrz")
                ghn = ps_n.tile([128, 2 * BC], F32, tag="ghn")
                girz = gi_cur[:, tloc, 0:4, :]
                ginn = gi_cur[:, tloc, 4:6, :]

                # PSUM preloads: gi(r,z) and bhn-replicated, via identity matmul
                nc.tensor.matmul(
                    ghrz[:, :], lhsT=ident[:, :], rhs=girz,
                    start=True, stop=False, skip_group_check=True,
                )
                nc.tensor.matmul(
                    ghn[:, :], lhsT=ident[:, :], rhs=bhnr_s[:, :],
                    start=True, stop=False, skip_group_check=True,
                )
                # recurrent matmuls: r,z chunks first (lets sigmoid start),
                # then n chunks
                for mc in range(4):
                    for k in range(2):
                        nc.tensor.matmul(
                            ghrz[:, mc * BC : (mc + 1) * BC],
                            lhsT=whb_s[:, k * 768 + mc * 128 : k * 768 + (mc + 1) * 128],
                            rhs=h_bf[:, k * BC : (k + 1) * BC],
                            start=False, stop=(k == 1), skip_group_check=True,
                        )
                rz_sig = gates.tile([128, 4 * BC], F32, tag="rzsig")
                nc.scalar.activation(rz_sig[:], ghrz[:], AF.Sigmoid)
                for mc in (4, 5):
                    for k in range(2):
                        nc.tensor.matmul(
                            ghn[:, (mc - 4) * BC : (mc - 3) * BC],
                            lhsT=whb_s[:, k * 768 + mc * 128 : k * 768 + (mc + 1) * 128],
                            rhs=h_bf[:, k * BC : (k + 1) * BC],
                            start=False, stop=(k == 1), skip_group_check=True,
                        )
                # projection of ys[t-1] (h_bf before this step's update):
                # off the critical path, runs on PE during the gate chain
                if t > 0:
                    proj_prev(t)
                # gi precompute for the next chunk, emitted here so the PE
                # stays warm during the gate chain
                stride = max(1, tc // GI_PIECES)
                if (
                    t // tc + 1 < n_chunks
                    and tloc % stride == 0
                    and tloc // stride < GI_PIECES
                ):
                    emit_gi_piece(t // tc + 1, tloc // stride)
                # DVE gate algebra (r = rz_sig[:, :2BC], z = rz_sig[:, 2BC:])
                z_ap = rz_sig[:, 2 * BC : 4 * BC]
                r_ap = rz_sig[:, 0 : 2 * BC]
                m1 = gates.tile([128, 2 * BC], F32, tag="m1")
                nc.vector.tensor_tensor(m1[:], ghn[:, :], r_ap, OP.mult)
                m2 = gates.tile([128, 2 * BC], F32, tag="m2")
                nc.vector.tensor_tensor(m2[:], m1[:], ginn, OP.add)
                n_act = gates.tile([128, 2 * BC], F32, tag="nact")
                nc.scalar.activation(n_act[:], m2[:], AF.Tanh)
                f1 = gates.tile([128, 2 * BC], F32, tag="f1")
                nc.vector.scalar_tensor_tensor(
                    f1[:], in0=z_ap, scalar=1.0, in1=n_act[:],
                    op0=OP.subtract, op1=OP.mult,
                )
                # v emitted late so it fills the DVE pipe between f1 and h_bf
                # instead of delaying m2 in the sigmoid->tanh window
                v = gates.tile([128, 2 * BC], F32, tag="v")
                nc.vector.tensor_tensor(v[:], z_ap, hT[:], OP.mult)
                # h_bf straight from (v, f1) so the PE unblocks one DVE op
                # earlier; the fp32 state update follows off the critical path
                nc.vector.tensor_tensor(h_bf[:], v[:], f1[:], OP.subtract)
                nc.vector.tensor_tensor(hT[:], v[:], f1[:], OP.subtract)

            # gi for chunk 0, then the scan with gi(c+1) interleaved
            for idx in range(GI_PIECES):
                emit_gi_piece(0, idx)
            stride = max(1, tc // GI_PIECES)
            for c in range(n_chunks):
                for tloc in range(tc):
                    emit_step(c * tc + tloc)
            # epilogue: project the last state, flush a partial output block
            proj_prev(T)
            if (T - 1) % 64 != 63:
                blk0 = ((T - 1) // 64) * 64
                ysb_f = gates.tile([1, 512], F32, tag="ysb", name="ysb_f")
                nc.vector.tensor_copy(
                    ysb_f[:, 0 : (T - blk0) * 8],
                    state["out_ps"][:, 0 : (T - blk0) * 8],
                )
                nc.sync.dma_start(
                    y_d[0:1, blk0 * 8 : T * 8],
                    ysb_f[:, 0 : (T - blk0) * 8],
                )

    nc.finalize()
    return nc


_PROGRAM_CACHE = {}


def get_program(T=T_FULL):
    if T not in _PROGRAM_CACHE:
        _PROGRAM_CACHE[T] = build_program(T, tc=min(128, T))
    return _PROGRAM_CACHE[T]


def make_in_maps(inputs, T=T_FULL):
    dur = np.asarray(inputs["duration_input"], np.float32)
    sid = np.asarray(inputs["sid_input"]).astype(np.int64)
    embed = np.asarray(inputs["embed"], np.float32)
    feats = np.concatenate([dur[..., None], embed[sid]], axis=-1)  # [B, T, 64]

    in_maps = []
    for c in range(NCORES):
        d = "f" if c < 4 else "b"
        s = c % 4
        f = feats[s * BC : (s + 1) * BC]  # [BC, T, 64]
        if d == "b":
            f = f[:, ::-1]
        fT = np.ascontiguousarray(f.transpose(2, 1, 0).reshape(FEAT, T * BC))
        Wh = np.asarray(inputs[f"Wh_{d}"], np.float32)  # [256, 768]
        Wi = np.asarray(inputs[f"Wi_{d}"], np.float32)  # [64, 768]
        bi = np.asarray(inputs[f"bi_{d}"], np.float32)  # [768]
        bhn = np.asarray(inputs[f"bhn_{d}"], np.float32)  # [256]
        Wd = np.asarray(inputs["Wd"], np.float32)[:, 0]  # [512]
        wd_half = Wd[:H] if d == "f" else Wd[H:]
        in_maps.append(
            {
                "featsT": fT.astype(NPBF16),
                "whb": np.ascontiguousarray(
                    Wh.reshape(2, 128, 768).transpose(1, 0, 2).reshape(128, 1536)
                ).astype(NPBF16),
                "wib": Wi.astype(NPBF16),
                "bi6": np.ascontiguousarray(bi.reshape(6, 128).T),
                "bhnr": np.ascontiguousarray(
                    np.repeat(bhn.reshape(2, 128).T, BC, axis=1)
                ).astype(NPBF16),
                "wd2": np.ascontiguousarray(wd_half.reshape(2, 128).T).astype(NPBF16),
            }
        )
    return in_maps


def assemble_output(results, inputs, T=T_FULL):
    parts = []
    for c in range(NCORES):
        y = np.asarray(results[c]["y"]).reshape(T, BC)  # [t, b]
        if c >= 4:
            y = y[::-1]
        parts.append(y.T)  # [BC, T]
    fwd = np.concatenate(parts[:4], axis=0)  # [B, T]
    bwd = np.concatenate(parts[4:], axis=0)
    bd = np.asarray(inputs["bd"], np.float32).reshape(-1)[0]
    return (fwd + bwd + bd)[..., None].astype(np.float32)


def kernel(**inputs):
    nc = get_program(T_FULL)
    in_maps = make_in_maps(inputs, T_FULL)
    res = run_bass_kernel_spmd(nc, in_maps, list(range(NCORES)))
    return assemble_output(res.results, inputs, T_FULL)
